# revision 2
# baseline (speedup 1.0000x reference)
"""TRN2 Bass kernel for nn_CrossModalAttention_75316546503126.

Mathematical collapse exploited here (verified against the jax reference):
K/V rows of the attention are identical across the sequence axis because the
acoustic features are broadcast before the K/V projections.  Hence every
attention row sees a constant score vector, softmax is exactly uniform
(S = 2048 is a power of two, so 1/S is exact in fp32), and

    attn_out[b, s, :] = v_b          with  v_b = (ac_b @ Wa + ba) @ Wv + bv
    out[b, s, :]      = text[b, s, :] @ Wt + (bt + v_b)

i.e. one [S, D] x [D, D] matmul per batch plus a per-batch bias row.
Q/K projections cancel entirely.

Sharding: data-parallel over batch B=8 across the 8 NeuronCores (core b
owns batch b).  Inside each core the [2048, 768] @ [768, 768] matmul runs
on the PE array in fp32r (fp32 with 12-bit mantissa, full PE rate), with
X 128x128 blocks transposed on-chip via PE transpose-mode; the bias row is
computed on-device and broadcast to all partitions with a tiny ones-matmul,
then folded into the PSUM->SBUF eviction add.

MODE:
  "f32r"   - single-pass fp32r matmul (max-rel-err ~1.4e-4 vs fp64)
  "split3" - hi/lo fp32r decomposition, 3 accumulated products
             (error ~1e-6, i.e. fp32-grade), ~2.2x the PE work
"""
import sys

if "/opt/trn_rl_repo" not in sys.path:
    sys.path.insert(0, "/opt/trn_rl_repo")

from contextlib import ExitStack

import numpy as np

import concourse.bacc as bacc
import concourse.bass as bass
import concourse.mybir as mybir
import concourse.tile as tile
from concourse.masks import make_identity
from concourse.bass_utils import run_bass_kernel_spmd

F32 = mybir.dt.float32
F32R = mybir.dt.float32r

B, S, D = 8, 2048, 768
KB = D // 128          # 6 contraction blocks
ST = S // 128          # 16 sequence tiles per core
N_CORES = 8

MODE = "f32r"


def build_program(mode=MODE):
    nc = bacc.Bacc()

    x = nc.declare_dram_parameter("x", [S, D], F32, isOutput=False)
    ac = nc.declare_dram_parameter("ac", [1, 16], F32, isOutput=False)
    wt = nc.declare_dram_parameter("wt", [D, D], F32, isOutput=False)
    wa = nc.declare_dram_parameter("wa", [16, D], F32, isOutput=False)
    wv = nc.declare_dram_parameter("wv", [D, D], F32, isOutput=False)
    bt = nc.declare_dram_parameter("bt", [D], F32, isOutput=False)
    ba = nc.declare_dram_parameter("ba", [D], F32, isOutput=False)
    bv = nc.declare_dram_parameter("bv", [D], F32, isOutput=False)
    out = nc.declare_dram_parameter("out", [S, D], F32, isOutput=True)

    with tile.TileContext(nc) as tc, ExitStack() as ctx:
        const = ctx.enter_context(tc.tile_pool(name="const", bufs=1))
        wpool = ctx.enter_context(tc.tile_pool(name="wpool", bufs=1))
        xpool = ctx.enter_context(tc.tile_pool(name="xpool", bufs=3))
        xtpool = ctx.enter_context(tc.tile_pool(name="xtpool", bufs=2))
        opool = ctx.enter_context(tc.tile_pool(name="opool", bufs=3))
        pst = ctx.enter_context(tc.tile_pool(name="pst", bufs=2, space="PSUM"))
        pso = ctx.enter_context(tc.tile_pool(name="pso", bufs=2, space="PSUM"))

        ident = const.tile([128, 128], F32)
        make_identity(nc, ident[:])

        # ---------------- weight load + fp32r rounding ----------------
        w_stage = wpool.tile([128, KB * D], F32, tag="wstage")
        w_hi = wpool.tile([128, KB * D], F32R, tag="whi")
        w_lo = None
        if mode == "split3":
            w_lo = wpool.tile([128, KB * D], F32R, tag="wlo")
        for k in range(KB):
            blk = slice(k * D, (k + 1) * D)
            nc.sync.dma_start(w_stage[:, blk], wt[k * 128:(k + 1) * 128, :])
            nc.vector.tensor_copy(w_hi[:, blk], w_stage[:, blk])
            if mode == "split3":
                lo_f = xpool.tile([128, D], F32, tag="wlof")
                nc.vector.tensor_sub(lo_f[:], w_stage[:, blk], w_hi[:, blk].bitcast(F32))
                nc.vector.tensor_copy(w_lo[:, blk], lo_f[:])

        # ---------------- acoustic path: v_b and the bias row ----------------
        # fa^T = (ac @ Wa)^T computed 128 rows at a time: out[m] = Wa[:,m]^T @ ac^T
        ac_sb = const.tile([16, 1], F32)
        nc.sync.dma_start(ac_sb[:, :], ac.rearrange("o k -> k o"))
        wa_sb = const.tile([16, D], F32)
        nc.sync.dma_start(wa_sb[:, :], wa[:])
        baT_sb = const.tile([128, KB], F32)
        nc.sync.dma_start(baT_sb[:, :], ba.rearrange("(m p) -> p m", p=128))

        fa_ps = pst.tile([128, KB * 128], F32, tag="tp")
        for m in range(KB):
            nc.tensor.matmul(
                fa_ps[:, m:m + 1],
                wa_sb[:, m * 128:(m + 1) * 128],
                ac_sb[:, :],
                start=True, stop=True,
            )
        faT_sb = const.tile([128, KB], F32)
        nc.vector.tensor_add(faT_sb[:], fa_ps[:, 0:KB], baT_sb[:])

        # Wv resident (needed once, for v = fa @ Wv)
        wv_sb = wpool.tile([128, KB * D], F32, tag="wstage2")
        for k in range(KB):
            nc.sync.dma_start(wv_sb[:, k * D:(k + 1) * D], wv[k * 128:(k + 1) * 128, :])

        # v row [1, 768] = fa @ Wv  (plain fp32 for full precision; tiny)
        v_ps = pst.tile([128, KB * 128], F32, tag="tp")
        for k in range(KB):
            nc.tensor.matmul(
                v_ps[0:1, 0:512],
                faT_sb[:, k:k + 1],
                wv_sb[:, k * D:k * D + 512],
                start=(k == 0), stop=(k == KB - 1),
            )
            nc.tensor.matmul(
                v_ps[0:1, 512:768],
                faT_sb[:, k:k + 1],
                wv_sb[:, k * D + 512:(k + 1) * D],
                start=(k == 0), stop=(k == KB - 1),
            )

        # bias row = bt + v + bv (all at partition 0), broadcast via K=1 matmul
        bt_row = const.tile([1, D], F32)
        bv_row = const.tile([1, D], F32)
        nc.sync.dma_start(bt_row[:, :], bt.rearrange("(o n) -> o n", o=1))
        nc.sync.dma_start(bv_row[:, :], bv.rearrange("(o n) -> o n", o=1))
        bias_row = const.tile([1, D], F32)
        nc.vector.tensor_add(bias_row[:], bt_row[:], bv_row[:])
        nc.vector.tensor_add(bias_row[:], bias_row[:], v_ps[0:1, 0:D])
        ones1 = const.tile([1, 128], F32)
        nc.gpsimd.memset(ones1[:], 1.0)

        bias_ps = pst.tile([128, KB * 128], F32, tag="tp")
        nc.tensor.matmul(bias_ps[:, 0:512], ones1[:], bias_row[:, 0:512],
                         start=True, stop=True)
        nc.tensor.matmul(bias_ps[:, 512:768], ones1[:], bias_row[:, 512:768],
                         start=True, stop=True)
        bias_sb = const.tile([128, D], F32)
        nc.vector.tensor_copy(bias_sb[:], bias_ps[:, 0:D])

        # ---------------- main loop over 16 sequence tiles ----------------
        for i in range(ST):
            rows = slice(i * 128, (i + 1) * 128)
            x_nat = xpool.tile([128, D], F32, tag="xnat")
            nc.sync.dma_start(x_nat[:], x[rows, :])

            # PE transpose of the 6 128x128 blocks -> PSUM
            tp = pst.tile([128, KB * 128], F32, tag="tp")
            for k in range(KB):
                blk = slice(k * 128, (k + 1) * 128)
                nc.tensor.transpose(tp[:, blk], x_nat[:, blk], ident[:])

            # rounded copyback to SBUF (f32r); split3 adds the residual
            xT = xtpool.tile([128, D], F32R, tag="xT")
            nc.vector.tensor_copy(xT[:, 0:512], tp[:, 0:512])
            nc.vector.tensor_copy(xT[:, 512:768], tp[:, 512:768])
            if mode == "split3":
                lo_f = xpool.tile([128, D], F32, tag="xlof")
                nc.vector.tensor_sub(lo_f[:], tp[:, 0:D], xT[:].bitcast(F32))
                xT_lo = xtpool.tile([128, D], F32R, tag="xTlo")
                nc.vector.tensor_copy(xT_lo[:], lo_f[:])

            # accumulated matmuls into [128, 768] PSUM (two banks: 512 + 256)
            ops = pso.tile([128, KB * 128], F32, tag="po")
            for lo_col, hi_col in ((0, 512), (512, 768)):
                if mode == "split3":
                    n_terms = 3 * KB
                    t = 0
                    for k in range(KB):
                        xblk = slice(k * 128, (k + 1) * 128)
                        for xa, wb in ((xT, w_hi), (xT, w_lo), (xT_lo, w_hi)):
                            nc.tensor.matmul(
                                ops[:, lo_col:hi_col],
                                xa[:, xblk],
                                wb[:, k * D + lo_col:k * D + hi_col],
                                start=(t == 0), stop=(t == n_terms - 1),
                            )
                            t += 1
                else:
                    for k in range(KB):
                        xblk = slice(k * 128, (k + 1) * 128)
                        nc.tensor.matmul(
                            ops[:, lo_col:hi_col],
                            xT[:, xblk],
                            w_hi[:, k * D + lo_col:k * D + hi_col],
                            start=(k == 0), stop=(k == KB - 1),
                        )

            out_sb = opool.tile([128, D], F32, tag="osb")
            nc.vector.tensor_add(out_sb[:], ops[:, 0:D], bias_sb[:])
            nc.sync.dma_start(out[rows, :], out_sb[:])

    nc.compile()
    return nc


_PROGRAM_CACHE = {}


def _get_program(mode=MODE):
    if mode not in _PROGRAM_CACHE:
        _PROGRAM_CACHE[mode] = build_program(mode)
    return _PROGRAM_CACHE[mode]


def kernel(text_features, acoustic_features, Wt, bt, Wa, ba, Wq, bq, Wk, bk,
           Wv, bv, **_unused):
    text_features = np.ascontiguousarray(np.asarray(text_features, dtype=np.float32))
    acoustic_features = np.ascontiguousarray(np.asarray(acoustic_features, dtype=np.float32))
    shared = {
        "wt": np.ascontiguousarray(np.asarray(Wt, dtype=np.float32)),
        "wa": np.ascontiguousarray(np.asarray(Wa, dtype=np.float32)),
        "wv": np.ascontiguousarray(np.asarray(Wv, dtype=np.float32)),
        "bt": np.ascontiguousarray(np.asarray(bt, dtype=np.float32)),
        "ba": np.ascontiguousarray(np.asarray(ba, dtype=np.float32)),
        "bv": np.ascontiguousarray(np.asarray(bv, dtype=np.float32)),
    }
    nc = _get_program()

    in_maps = []
    for b in range(N_CORES):
        m = dict(shared)
        m["x"] = text_features[b]
        m["ac"] = acoustic_features[b:b + 1]
        in_maps.append(m)

    res = run_bass_kernel_spmd(nc, in_maps, list(range(N_CORES))).results
    out = np.empty((B, S, D), dtype=np.float32)
    for b in range(N_CORES):
        out[b] = res[b]["out"]
    return out


# revision 3
# speedup vs baseline: 1.0309x; 1.0309x over previous
"""TRN2 Bass kernel for nn_CrossModalAttention_75316546503126.

Mathematical collapse exploited here (verified against the jax reference):
K/V rows of the attention are identical across the sequence axis because the
acoustic features are broadcast before the K/V projections.  Hence every
attention row sees a constant score vector, softmax is exactly uniform
(S = 2048 is a power of two, so 1/S is exact in fp32), and

    attn_out[b, s, :] = v_b          with  v_b = (ac_b @ Wa + ba) @ Wv + bv
    out[b, s, :]      = text[b, s, :] @ Wt + (bt + v_b)

i.e. one [S, D] x [D, D] matmul per batch plus a per-batch bias row.
Q/K projections cancel entirely.

Sharding: data-parallel over batch B=8 across the 8 NeuronCores (core b
owns batch b).  Inside each core the [2048, 768] @ [768, 768] matmul runs
on the PE array in fp32r (fp32 with 12-bit mantissa, full PE rate), with
X 128x128 blocks transposed on-chip via PE transpose-mode; the bias row is
computed on-device and broadcast to all partitions with a tiny ones-matmul,
then folded into the PSUM->SBUF eviction add.  Sequence tiles are processed
in superblocks of 4 so the PE sees long dense matmul bursts (keeps the HAM
clock-gate at 2.4 GHz).

MODE:
  "f32r"   - single-pass fp32r matmul (max-rel-err ~1.3e-4 vs fp64)
  "split3" - hi/lo fp32r decomposition, 3 accumulated products
             (error ~1e-6, i.e. fp32-grade), ~2.2x the PE work
"""
import sys

if "/opt/trn_rl_repo" not in sys.path:
    sys.path.insert(0, "/opt/trn_rl_repo")

from contextlib import ExitStack

import numpy as np

import concourse.bacc as bacc
import concourse.bass as bass
import concourse.mybir as mybir
import concourse.tile as tile
from concourse.masks import make_identity
from concourse.bass_utils import run_bass_kernel_spmd

F32 = mybir.dt.float32
F32R = mybir.dt.float32r

B, S, D = 8, 2048, 768
KB = D // 128          # 6 contraction blocks
ST = S // 128          # 16 sequence tiles per core
SB = 4                 # sequence tiles per superblock
N_CORES = 8

MODE = "f32r"


def build_program(mode=MODE):
    nc = bacc.Bacc()

    x = nc.declare_dram_parameter("x", [S, D], F32, isOutput=False)
    ac = nc.declare_dram_parameter("ac", [1, 16], F32, isOutput=False)
    wt = nc.declare_dram_parameter("wt", [D, D], F32, isOutput=False)
    wa = nc.declare_dram_parameter("wa", [16, D], F32, isOutput=False)
    wv = nc.declare_dram_parameter("wv", [D, D], F32, isOutput=False)
    bt = nc.declare_dram_parameter("bt", [D], F32, isOutput=False)
    ba = nc.declare_dram_parameter("ba", [D], F32, isOutput=False)
    bv = nc.declare_dram_parameter("bv", [D], F32, isOutput=False)
    out = nc.declare_dram_parameter("out", [S, D], F32, isOutput=True)

    split3 = mode == "split3"

    with tile.TileContext(nc) as tc, ExitStack() as ctx:
        const = ctx.enter_context(tc.tile_pool(name="const", bufs=1))
        wpool = ctx.enter_context(tc.tile_pool(name="wpool", bufs=1))
        xpool = ctx.enter_context(tc.tile_pool(name="xpool", bufs=4))
        xtpool = ctx.enter_context(tc.tile_pool(name="xtpool", bufs=SB + 2))
        opool = ctx.enter_context(tc.tile_pool(name="opool", bufs=3))
        pst = ctx.enter_context(tc.tile_pool(name="pst", bufs=2, space="PSUM"))
        pso = ctx.enter_context(tc.tile_pool(name="pso", bufs=2, space="PSUM"))

        ident = const.tile([128, 128], F32)
        make_identity(nc, ident[:])

        # ---------------- acoustic path inputs (tiny, load first) -------------
        ac_sb = const.tile([16, 1], F32)
        nc.sync.dma_start(ac_sb[:, :], ac.rearrange("o k -> k o"))
        wa_sb = const.tile([16, D], F32)
        nc.sync.dma_start(wa_sb[:, :], wa[:])
        baT_sb = const.tile([128, KB], F32)
        nc.sync.dma_start(baT_sb[:, :], ba.rearrange("(m p) -> p m", p=128))
        bt_row = const.tile([1, D], F32)
        bv_row = const.tile([1, D], F32)
        nc.sync.dma_start(bt_row[:, :], bt.rearrange("(o n) -> o n", o=1))
        nc.sync.dma_start(bv_row[:, :], bv.rearrange("(o n) -> o n", o=1))

        # ---------------- weight load + fp32r rounding ----------------
        w_stage = wpool.tile([128, KB * D], F32, tag="wstage")
        w_hi = wpool.tile([128, KB * D], F32R, tag="whi")
        w_lo = wpool.tile([128, KB * D], F32R, tag="wlo") if split3 else None
        for k in range(KB):
            blk = slice(k * D, (k + 1) * D)
            nc.sync.dma_start(w_stage[:, blk], wt[k * 128:(k + 1) * 128, :])
            nc.vector.tensor_copy(w_hi[:, blk], w_stage[:, blk])
            if split3:
                lo_f = xpool.tile([128, D], F32, tag="wlof")
                nc.vector.tensor_sub(lo_f[:], w_stage[:, blk], w_hi[:, blk].bitcast(F32))
                nc.vector.tensor_copy(w_lo[:, blk], lo_f[:])

        # Wv resident (used once for v = fa @ Wv), rounded to f32r
        wv_stage = wpool.tile([128, KB * D], F32, tag="wstage2")
        wv_hi = wpool.tile([128, KB * D], F32R, tag="wvhi")
        wv_lo = wpool.tile([128, KB * D], F32R, tag="wvlo") if split3 else None
        for k in range(KB):
            blk = slice(k * D, (k + 1) * D)
            nc.sync.dma_start(wv_stage[:, blk], wv[k * 128:(k + 1) * 128, :])
            nc.vector.tensor_copy(wv_hi[:, blk], wv_stage[:, blk])
            if split3:
                lo_f = xpool.tile([128, D], F32, tag="wlof")
                nc.vector.tensor_sub(lo_f[:], wv_stage[:, blk], wv_hi[:, blk].bitcast(F32))
                nc.vector.tensor_copy(wv_lo[:, blk], lo_f[:])

        # ---------------- fa^T = (ac @ Wa + ba)^T  (plain fp32, tiny) ---------
        fa_ps = pst.tile([128, KB * 128], F32, tag="tp")
        for m in range(KB):
            nc.tensor.matmul(
                fa_ps[:, m:m + 1],
                wa_sb[:, m * 128:(m + 1) * 128],
                ac_sb[:, :],
                start=True, stop=True,
            )
        faT_sb = const.tile([128, KB], F32)
        nc.vector.tensor_add(faT_sb[:], fa_ps[:, 0:KB], baT_sb[:])
        faT_hi = const.tile([128, KB], F32R)
        nc.vector.tensor_copy(faT_hi[:], faT_sb[:])
        if split3:
            faT_lof = const.tile([128, KB], F32)
            nc.vector.tensor_sub(faT_lof[:], faT_sb[:], faT_hi[:].bitcast(F32))
            faT_lo = const.tile([128, KB], F32R)
            nc.vector.tensor_copy(faT_lo[:], faT_lof[:])

        # ---------------- v row [1, 768] = fa @ Wv (f32r) ----------------
        v_ps = pst.tile([128, KB * 128], F32, tag="tp")
        if split3:
            terms = ((faT_hi, wv_hi), (faT_hi, wv_lo), (faT_lo, wv_hi))
        else:
            terms = ((faT_hi, wv_hi),)
        for lo_col, hi_col in ((0, 512), (512, 768)):
            t, ntot = 0, KB * len(terms)
            for k in range(KB):
                for fv, wvp in terms:
                    nc.tensor.matmul(
                        v_ps[0:1, lo_col:hi_col],
                        fv[:, k:k + 1],
                        wvp[:, k * D + lo_col:k * D + hi_col],
                        start=(t == 0), stop=(t == ntot - 1),
                    )
                    t += 1

        # bias row = bt + v + bv at partition 0, broadcast via K=1 fp32 matmul
        bias_row = const.tile([1, D], F32)
        nc.vector.tensor_add(bias_row[:], bt_row[:], bv_row[:])
        nc.vector.tensor_add(bias_row[:], bias_row[:], v_ps[0:1, 0:D])
        ones1 = const.tile([1, 128], F32)
        nc.gpsimd.memset(ones1[:], 1.0)

        bias_ps = pst.tile([128, KB * 128], F32, tag="tp")
        nc.tensor.matmul(bias_ps[:, 0:512], ones1[:], bias_row[:, 0:512],
                         start=True, stop=True)
        nc.tensor.matmul(bias_ps[:, 512:768], ones1[:], bias_row[:, 512:768],
                         start=True, stop=True)
        bias_sb = const.tile([128, D], F32)
        nc.vector.tensor_copy(bias_sb[:], bias_ps[:, 0:D])

        # ---------------- main loop: superblocks of SB sequence tiles ---------
        for sb in range(ST // SB):
            xTs = []
            # transpose phase: SB tiles of X, PE-transposed and rounded to f32r
            for j in range(SB):
                i = sb * SB + j
                rows = slice(i * 128, (i + 1) * 128)
                x_nat = xpool.tile([128, D], F32, tag="xnat")
                nc.sync.dma_start(x_nat[:], x[rows, :])

                tp = pst.tile([128, KB * 128], F32, tag="tp")
                for k in range(KB):
                    blk = slice(k * 128, (k + 1) * 128)
                    nc.tensor.transpose(tp[:, blk], x_nat[:, blk], ident[:])

                xT = xtpool.tile([128, D], F32R, tag="xT")
                nc.vector.tensor_copy(xT[:], tp[:, 0:D])
                if split3:
                    lo_f = xpool.tile([128, D], F32, tag="xlof")
                    nc.vector.tensor_sub(lo_f[:], tp[:, 0:D], xT[:].bitcast(F32))
                    xT_lo = xtpool.tile([128, D], F32R, tag="xTlo")
                    nc.vector.tensor_copy(xT_lo[:], lo_f[:])
                    xTs.append((xT, xT_lo))
                else:
                    xTs.append((xT, None))

            # dense matmul burst over the superblock
            for j in range(SB):
                i = sb * SB + j
                rows = slice(i * 128, (i + 1) * 128)
                xT, xT_lo = xTs[j]
                ops = pso.tile([128, KB * 128], F32, tag="po")
                if split3:
                    prods = ((xT, w_hi), (xT, w_lo), (xT_lo, w_hi))
                else:
                    prods = ((xT, w_hi),)
                ntot = KB * len(prods)
                t = 0
                for k in range(KB):
                    xblk = slice(k * 128, (k + 1) * 128)
                    for xa, wb in prods:
                        st, sp = (t == 0), (t == ntot - 1)
                        nc.tensor.matmul(
                            ops[:, 0:512], xa[:, xblk],
                            wb[:, k * D:k * D + 512], start=st, stop=sp)
                        nc.tensor.matmul(
                            ops[:, 512:768], xa[:, xblk],
                            wb[:, k * D + 512:(k + 1) * D], start=st, stop=sp)
                        t += 1

                out_sb = opool.tile([128, D], F32, tag="osb")
                nc.vector.tensor_add(out_sb[:], ops[:, 0:D], bias_sb[:])
                nc.sync.dma_start(out[rows, :], out_sb[:])

    nc.compile()
    return nc


_PROGRAM_CACHE = {}


def _get_program(mode=MODE):
    if mode not in _PROGRAM_CACHE:
        _PROGRAM_CACHE[mode] = build_program(mode)
    return _PROGRAM_CACHE[mode]


def kernel(text_features, acoustic_features, Wt, bt, Wa, ba, Wq, bq, Wk, bk,
           Wv, bv, **_unused):
    text_features = np.ascontiguousarray(np.asarray(text_features, dtype=np.float32))
    acoustic_features = np.ascontiguousarray(np.asarray(acoustic_features, dtype=np.float32))
    shared = {
        "wt": np.ascontiguousarray(np.asarray(Wt, dtype=np.float32)),
        "wa": np.ascontiguousarray(np.asarray(Wa, dtype=np.float32)),
        "wv": np.ascontiguousarray(np.asarray(Wv, dtype=np.float32)),
        "bt": np.ascontiguousarray(np.asarray(bt, dtype=np.float32)),
        "ba": np.ascontiguousarray(np.asarray(ba, dtype=np.float32)),
        "bv": np.ascontiguousarray(np.asarray(bv, dtype=np.float32)),
    }
    nc = _get_program()

    in_maps = []
    for b in range(N_CORES):
        m = dict(shared)
        m["x"] = text_features[b]
        m["ac"] = acoustic_features[b:b + 1]
        in_maps.append(m)

    res = run_bass_kernel_spmd(nc, in_maps, list(range(N_CORES))).results
    out = np.empty((B, S, D), dtype=np.float32)
    for b in range(N_CORES):
        out[b] = res[b]["out"]
    return out


# revision 9
# speedup vs baseline: 1.0548x; 1.0232x over previous
"""TRN2 Bass kernel for nn_CrossModalAttention_75316546503126.

Mathematical collapse exploited here (verified against the jax reference):
K/V rows of the attention are identical across the sequence axis because the
acoustic features are broadcast before the K/V projections.  Hence every
attention row sees a constant score vector, softmax is exactly uniform
(S = 2048 is a power of two, so 1/S is exact in fp32), and

    attn_out[b, s, :] = v_b          with  v_b = (ac_b @ Wa + ba) @ Wv + bv
    out[b, s, :]      = text[b, s, :] @ Wt + (bt + v_b)

i.e. one [S, D] x [D, D] matmul per batch plus a per-batch bias row.
Q/K projections cancel entirely.

Sharding: data-parallel over batch B=8 across the 8 NeuronCores (core b
owns batch b).  Inside each core the [2048, 768] @ [768, 768] matmul runs
on the PE array in fp32r (fp32 with 12-bit mantissa, full PE rate), with
X 128x128 blocks transposed on-chip via PE transpose-mode; the bias row is
computed on-device and broadcast to all partitions with a tiny ones-matmul,
then folded into the PSUM->SBUF eviction add.  Sequence tiles are processed
in superblocks of 4 so the PE sees long dense matmul bursts (keeps the HAM
clock-gate at 2.4 GHz).

MODE:
  "f32r"   - single-pass fp32r matmul (max-rel-err ~1.6e-4 vs fp64)
  "split3" - hi/lo fp32r decomposition, 3 accumulated products
             (error ~1e-6, i.e. fp32-grade), ~2.2x the PE work
"""
import sys

if "/opt/trn_rl_repo" not in sys.path:
    sys.path.insert(0, "/opt/trn_rl_repo")

from contextlib import ExitStack

import numpy as np

import concourse.bacc as bacc
import concourse.bass as bass
import concourse.mybir as mybir
import concourse.tile as tile
from concourse.masks import make_identity
from concourse.bass_utils import run_bass_kernel_spmd

F32 = mybir.dt.float32
F32R = mybir.dt.float32r

B, S, D = 8, 2048, 768
KB = D // 128          # 6 contraction blocks
ST = S // 128          # 16 sequence tiles per core
SB = 4                 # sequence tiles per superblock
N_CORES = 8

MODE = "f32r"


def build_program(mode=MODE):
    nc = bacc.Bacc()

    x = nc.declare_dram_parameter("x", [S, D], F32, isOutput=False)
    ac = nc.declare_dram_parameter("ac", [1, 16], F32, isOutput=False)
    wt = nc.declare_dram_parameter("wt", [D, D], F32, isOutput=False)
    wa = nc.declare_dram_parameter("wa", [16, D], F32, isOutput=False)
    wv = nc.declare_dram_parameter("wv", [D, D], F32, isOutput=False)
    bt = nc.declare_dram_parameter("bt", [D], F32, isOutput=False)
    ba = nc.declare_dram_parameter("ba", [D], F32, isOutput=False)
    bv = nc.declare_dram_parameter("bv", [D], F32, isOutput=False)
    out = nc.declare_dram_parameter("out", [S, D], F32, isOutput=True)

    split3 = mode == "split3"

    with tile.TileContext(nc) as tc, ExitStack() as ctx:
        const = ctx.enter_context(tc.tile_pool(name="const", bufs=1))
        wpool = ctx.enter_context(tc.tile_pool(name="wpool", bufs=1))
        xpool = ctx.enter_context(tc.tile_pool(name="xpool", bufs=4))
        xtpool = ctx.enter_context(tc.tile_pool(name="xtpool", bufs=SB + 2))
        opool = ctx.enter_context(tc.tile_pool(name="opool", bufs=6))
        # PSUM budget (8 banks): transposes 3x[128,512] = 3, out 2x[128,768] = 4,
        # setup 1x[128,512] = 1
        pst = ctx.enter_context(tc.tile_pool(name="pst", bufs=3, space="PSUM"))
        pso = ctx.enter_context(tc.tile_pool(name="pso", bufs=2, space="PSUM"))
        pset = ctx.enter_context(tc.tile_pool(name="pset", bufs=1, space="PSUM"))

        ident = const.tile([128, 128], F32)
        make_identity(nc, ident[:])

        # ---------------- tiny inputs ----------------
        ac_sb = const.tile([16, 1], F32)
        nc.sync.dma_start(ac_sb[:, :], ac.rearrange("o k -> k o"))
        wa_sb = const.tile([16, D], F32)
        nc.sync.dma_start(wa_sb[:, :], wa[:])
        baT_sb = const.tile([128, KB], F32)
        nc.sync.dma_start(baT_sb[:, :], ba.rearrange("(m p) -> p m", p=128))
        bt_row = const.tile([1, D], F32)
        bv_row = const.tile([1, D], F32)
        nc.sync.dma_start(bt_row[:, :], bt.rearrange("(o n) -> o n", o=1))
        nc.sync.dma_start(bv_row[:, :], bv.rearrange("(o n) -> o n", o=1))

        # ---------------- helpers ----------------
        w_hi, w_lo, wv_hi, wv_lo = [], [], [], []

        def _round_block(src_dram, row0, hi_list, lo_list, nm):
            stage = xpool.tile([128, D], F32, tag="wstg")
            nc.sync.dma_start(stage[:], src_dram[row0:row0 + 128, :])
            hi = wpool.tile([128, D], F32R, tag=f"{nm}hi{row0}")
            nc.vector.tensor_copy(hi[:], stage[:])
            hi_list.append(hi)
            if split3:
                lo_f = xpool.tile([128, D], F32, tag="wlof")
                nc.vector.tensor_sub(lo_f[:], stage[:], hi[:].bitcast(F32))
                lo = wpool.tile([128, D], F32R, tag=f"{nm}lo{row0}")
                nc.vector.tensor_copy(lo[:], lo_f[:])
                lo_list.append(lo)

        x_nats = {}

        def _xdma(i):
            x_nats[i] = xpool.tile([128, D], F32, tag="xnat", name=f"xnat{i}")
            nc.sync.dma_start(x_nats[i][:], x[i * 128:(i + 1) * 128, :])

        # first superblock's X + Wt first: they gate the first PE burst.
        for i in range(SB):
            _xdma(i)
        for k in range(KB):
            _round_block(wt, k * 128, w_hi, w_lo, "wt")

        # ---------------- fa^T = (ac @ Wa + ba)^T ----------------
        fa_in, wa_in = wa_sb, ac_sb           # plain fp32 (fp32r shape-illegal here)

        fa_ps = pset.tile([128, 512], F32, tag="setup")
        for m in range(KB):
            nc.tensor.matmul(
                fa_ps[:, m:m + 1],
                fa_in[:, m * 128:(m + 1) * 128],
                wa_in[:, :],
                start=True, stop=True,
            )
        faT_sb = const.tile([128, KB], F32)
        nc.vector.tensor_add(faT_sb[:], fa_ps[:, 0:KB], baT_sb[:])
        faT_hi = const.tile([128, KB], F32R)
        nc.vector.tensor_copy(faT_hi[:], faT_sb[:])
        if split3:
            faT_lof = const.tile([128, KB], F32)
            nc.vector.tensor_sub(faT_lof[:], faT_sb[:], faT_hi[:].bitcast(F32))
            faT_lo = const.tile([128, KB], F32R)
            nc.vector.tensor_copy(faT_lo[:], faT_lof[:])

        # ---------------- phase emitters for the main loop ----------------
        xTs = {}

        def emit_transpose_phase(sb):
            for j in range(SB):
                i = sb * SB + j
                x_nat = x_nats.pop(i)

                tpA = pst.tile([128, 512], F32, tag="tp")
                tpB = pst.tile([128, 512], F32, tag="tp")
                for k in range(KB):
                    blk = slice(k * 128, (k + 1) * 128)
                    if k < 4:
                        nc.tensor.transpose(tpA[:, k * 128:(k + 1) * 128],
                                            x_nat[:, blk], ident[:])
                    else:
                        nc.tensor.transpose(tpB[:, (k - 4) * 128:(k - 3) * 128],
                                            x_nat[:, blk], ident[:])

                xT = xtpool.tile([128, D], F32R, tag="xT")
                nc.vector.tensor_copy(xT[:, 0:512], tpA[:])
                nc.vector.tensor_copy(xT[:, 512:768], tpB[:, 0:256])
                if split3:
                    lo_fA = xpool.tile([128, 512], F32, tag="xlofA")
                    nc.vector.tensor_sub(lo_fA[:], tpA[:], xT[:, 0:512].bitcast(F32))
                    lo_fB = xpool.tile([128, 256], F32, tag="xlofB")
                    nc.vector.tensor_sub(lo_fB[:], tpB[:, 0:256],
                                         xT[:, 512:768].bitcast(F32))
                    xT_lo = xtpool.tile([128, D], F32R, tag="xTlo")
                    nc.vector.tensor_copy(xT_lo[:, 0:512], lo_fA[:])
                    nc.vector.tensor_copy(xT_lo[:, 512:768], lo_fB[:])
                    xTs[i] = (xT, xT_lo)
                else:
                    xTs[i] = (xT, None)

        def emit_burst(sb, bias_sb, defer):
            deferred = []
            for j in range(SB):
                i = sb * SB + j
                rows = slice(i * 128, (i + 1) * 128)
                xT, xT_lo = xTs.pop(i)
                ops = pso.tile([128, KB * 128], F32, tag="po")
                if split3:
                    prods = ((xT, w_hi), (xT, w_lo), (xT_lo, w_hi))
                else:
                    prods = ((xT, w_hi),)
                ntot = KB * len(prods)
                t = 0
                for k in range(KB):
                    xblk = slice(k * 128, (k + 1) * 128)
                    for xa, wbl in prods:
                        st, sp = (t == 0), (t == ntot - 1)
                        nc.tensor.matmul(
                            ops[:, 0:512], xa[:, xblk],
                            wbl[k][:, 0:512], start=st, stop=sp)
                        nc.tensor.matmul(
                            ops[:, 512:768], xa[:, xblk],
                            wbl[k][:, 512:768], start=st, stop=sp)
                        t += 1

                if defer:
                    # bias not computed yet (program order): copy out of PSUM
                    # now, add the bias once it exists (emit_deferred_bias)
                    raw = opool.tile([128, D], F32, tag="oraw")
                    nc.vector.tensor_copy(raw[:], ops[:, 0:D])
                    deferred.append((rows, raw))
                else:
                    out_sb = opool.tile([128, D], F32, tag="osb")
                    nc.vector.tensor_add(out_sb[:], ops[:, 0:D], bias_sb[:])
                    nc.sync.dma_start(out[rows, :], out_sb[:])
            return deferred

        def emit_deferred_bias(deferred, bias_sb):
            for rows, raw in deferred:
                out_sb = opool.tile([128, D], F32, tag="osb")
                nc.vector.tensor_add(out_sb[:], raw[:], bias_sb[:])
                nc.sync.dma_start(out[rows, :], out_sb[:])

        # ---------------- SB0 transposes, then Wv load, then SB0 burst --------
        emit_transpose_phase(0)
        for i in range(SB, 2 * SB):
            _xdma(i)
        for k in range(KB):
            _round_block(wv, k * 128, wv_hi, wv_lo, "wv")

        bias_sb = const.tile([128, D], F32)
        deferred0 = emit_burst(0, bias_sb, defer=True)

        # ---------------- v row + bias row + broadcast (after SB0 burst) ------
        bias_row = const.tile([1, D], F32)
        nc.vector.tensor_add(bias_row[:], bt_row[:], bv_row[:])
        if split3:
            vterms = ((faT_hi, wv_hi), (faT_hi, wv_lo), (faT_lo, wv_hi))
        else:
            vterms = ((faT_hi, wv_hi),)
        for lo_col, hi_col in ((0, 512), (512, 768)):
            v_ps = pset.tile([128, 512], F32, tag="setup")
            t, ntot = 0, KB * len(vterms)
            for k in range(KB):
                for fv, wvl in vterms:
                    nc.tensor.matmul(
                        v_ps[0:1, 0:hi_col - lo_col],
                        fv[:, k:k + 1],
                        wvl[k][:, lo_col:hi_col],
                        start=(t == 0), stop=(t == ntot - 1),
                    )
                    t += 1
            nc.vector.tensor_add(bias_row[:, lo_col:hi_col],
                                 bias_row[:, lo_col:hi_col],
                                 v_ps[0:1, 0:hi_col - lo_col])

        ones1 = const.tile([1, 128], F32)
        nc.gpsimd.memset(ones1[:], 1.0)
        bias_bc = bias_row
        for lo_col, hi_col in ((0, 512), (512, 768)):
            bias_ps = pset.tile([128, 512], F32, tag="setup")
            nc.tensor.matmul(bias_ps[:, 0:hi_col - lo_col], ones1[:],
                             bias_bc[:, lo_col:hi_col], start=True, stop=True)
            nc.vector.tensor_copy(bias_sb[:, lo_col:hi_col],
                                  bias_ps[:, 0:hi_col - lo_col])
        emit_deferred_bias(deferred0, bias_sb)

        # ---------------- remaining superblocks ----------------
        for sb in range(1, ST // SB):
            emit_transpose_phase(sb)
            if sb + 1 < ST // SB:
                for j in range(SB):
                    _xdma((sb + 1) * SB + j)
            emit_burst(sb, bias_sb, defer=False)

    nc.compile()
    return nc


_PROGRAM_CACHE = {}


def _get_program(mode=MODE):
    if mode not in _PROGRAM_CACHE:
        _PROGRAM_CACHE[mode] = build_program(mode)
    return _PROGRAM_CACHE[mode]


def kernel(text_features, acoustic_features, Wt, bt, Wa, ba, Wq, bq, Wk, bk,
           Wv, bv, **_unused):
    text_features = np.ascontiguousarray(np.asarray(text_features, dtype=np.float32))
    acoustic_features = np.ascontiguousarray(np.asarray(acoustic_features, dtype=np.float32))
    shared = {
        "wt": np.ascontiguousarray(np.asarray(Wt, dtype=np.float32)),
        "wa": np.ascontiguousarray(np.asarray(Wa, dtype=np.float32)),
        "wv": np.ascontiguousarray(np.asarray(Wv, dtype=np.float32)),
        "bt": np.ascontiguousarray(np.asarray(bt, dtype=np.float32)),
        "ba": np.ascontiguousarray(np.asarray(ba, dtype=np.float32)),
        "bv": np.ascontiguousarray(np.asarray(bv, dtype=np.float32)),
    }
    nc = _get_program()

    in_maps = []
    for b in range(N_CORES):
        m = dict(shared)
        m["x"] = text_features[b]
        m["ac"] = acoustic_features[b:b + 1]
        in_maps.append(m)

    res = run_bass_kernel_spmd(nc, in_maps, list(range(N_CORES))).results
    out = np.empty((B, S, D), dtype=np.float32)
    for b in range(N_CORES):
        out[b] = res[b]["out"]
    return out
